# revision 18
# baseline (speedup 1.0000x reference)
"""Expert-parallel MoE MLP (ExpertMLP) Bass kernel for 8 Trainium2 NeuronCores.

Problem: x[32,4096,256] @ w_fc[32,256,1024] -> gelu(erf) -> @ w_proj[32,1024,256].

Sharding: expert-parallel. Each of the 8 cores gets 4 experts (slices of the
leading axis of every tensor); no cross-core communication.

Per-core dataflow (PE-bound problem: 17.2 GFLOP/core, bf16 roofline ~219us):

  1. x[e] is cast fp32->bf16 DRAM->DRAM in 1024-row chunks on the gpsimd
     software-DGE queue, then XBar DMA-transposed chunk-by-chunk into SBUF
     xT tiles [d(128), 1024c]. Fine granularity lets the first MM1 start as
     soon as the first chunk's cast lands (~20us) instead of after the
     whole-tensor staging (~45us).
  2. ALL HWDGE DMAs (weights, transposes, output stores) ride the single
     SyncE queue: bass rotates HWDGE completions through 8 shared DMAHW
     semaphores and the thresholds are only sound when the DMAs retire in
     one FIFO order -- splitting across the Sync+Act HWDGE queues corrupts
     the counts (measured: wrong results + 92us serialization stalls).
  3. Weights stream in halves, cast fp32->bf16 by the mostly-idle VectorE,
     so MM1 of an expert only waits on the first half.
  4. MM1: hT[h_tile, c512] += w_fc_tile.T @ xT (stationary = w_fc natural
     [d,h] layout; moving N=512; 2 h_tiles packed per ps_h tile so GELU
     evicts in wide ACTIVATE calls).
  5. GELU (erf) runs on ACT as the PSUM->SBUF eviction, writing bf16 hT.
  6. MM2 is k-major: for each h_tile kk (as soon as its GELU lands), all 4
     capacity slices accumulate pso[:, s, :] += hT_kk.T @ w_proj[kk].
     MM2 overlaps MM1/GELU of the same chunk and the chunk-boundary PE
     bubble of the slice-major order disappears.
  7. pso (2 banks, double-buffered) is evicted by VectorE to an SBUF
     staging tile and DMA'd out per 512-row chunk.

PSUM budget: ps_h 2x[128,2,512]f32 (4 banks) + ps_o 2x[128,4,256]f32
(4 banks) = 8 banks exactly.
"""

import numpy as np
from contextlib import ExitStack

import bass_rust as _br
import concourse.bass as bass
import concourse.tile as tile
from concourse import mybir
from concourse.bass_utils import run_bass_kernel_spmd

E, CAP, D, H = 32, 4096, 256, 1024
N_CORES = 8
E_PER = E // N_CORES  # 4 experts per core
P = 128
F32 = mybir.dt.float32
BF16 = mybir.dt.bfloat16

KD = D // P        # 2 k-tiles in MM1's contraction
KH = H // P        # 8 k-tiles in MM2's contraction
NC_CHUNK = 512     # capacity chunk processed per MM1/MM2 round
N_CHUNKS = CAP // NC_CHUNK          # 8 per expert
CC_ROWS = 1024     # cast/transpose chunk (capacity rows)
N_CC = CAP // CC_ROWS               # 4 per expert
H_TILES = H // P   # 8
HPACK = 2          # h_tiles packed per ps_h tile / GELU call
S_PER = NC_CHUNK // P               # 4 capacity slices per chunk


def _fix_waits(nc):
    """walrus here accepts only one sync wait per instruction; hoist excess
    waits onto standalone EventSemaphore instructions inserted before the
    offender (same engine => same sequencer order)."""
    for fn in nc.m.functions:
        for bb in fn.blocks:
            new = []
            changed = False
            for inst in bb.instructions:
                si = inst.sync_info
                if si is not None and len(si.on_wait) > 1:
                    waits = list(si.on_wait)
                    for w in waits[:-1]:
                        ev = mybir.InstEventSemaphore(
                            name=nc.get_next_instruction_name()
                        )
                        ev.engine = inst.engine
                        ev.sync_info = _br.SyncInfo(on_wait=[w], on_update=[])
                        nc.register_instruction(ev)
                        new.append(ev)
                    inst.sync_info = _br.SyncInfo(
                        on_wait=waits[-1:], on_update=list(si.on_update)
                    )
                    changed = True
                new.append(inst)
            if changed:
                bb.instructions = new


def _build():
    nc = bass.Bass(trn_type="TRN2", target_bir_lowering=False, debug=False)
    x = nc.dram_tensor("x", [E_PER, CAP, D], F32, kind="ExternalInput").ap()
    w_fc = nc.dram_tensor("w_fc", [E_PER, D, H], F32, kind="ExternalInput").ap()
    w_proj = nc.dram_tensor("w_proj", [E_PER, H, D], F32, kind="ExternalInput").ap()
    out = nc.dram_tensor("out", [E_PER, CAP, D], F32, kind="ExternalOutput").ap()
    # bf16 staging of x in DRAM, one tensor per 1024-row chunk (16 total,
    # g = e*4+cc) so each XBar transpose depends only on its own cast chunk.
    # The first chunk is split in half so the very first transposes (and MM1)
    # start ~2.5us earlier.
    xbf0 = [nc.dram_tensor(f"xbf0{h}", [CC_ROWS // 2, D], BF16).ap() for h in range(2)]
    xbf = [None] + [
        nc.dram_tensor(f"xbf{g}", [CC_ROWS, D], BF16).ap() for g in range(1, 16)
    ]

    with tile.TileContext(nc) as tc, ExitStack() as ctx:
        xtp = ctx.enter_context(tc.tile_pool(name="xtp", bufs=E_PER * KD * N_CC))
        wload = ctx.enter_context(tc.tile_pool(name="wload", bufs=2))
        wfc_p = ctx.enter_context(tc.tile_pool(name="wfc", bufs=2))
        wproj_p = ctx.enter_context(tc.tile_pool(name="wproj", bufs=2))
        ht_p = ctx.enter_context(tc.tile_pool(name="ht", bufs=2 * H_TILES // HPACK))
        out_p = ctx.enter_context(tc.tile_pool(name="outp", bufs=3))
        ps_h = ctx.enter_context(tc.tile_pool(name="ps_h", bufs=2, space="PSUM"))
        ps_o = ctx.enter_context(tc.tile_pool(name="ps_o", bufs=2, space="PSUM"))

        def load_weights(e):
            """Weight DMAs in halves; VectorE casts each half to bf16 as it
            lands so the first MM1 group of an expert only waits on half 0."""
            wfc_raw = wload.tile([P, KD, H], F32, tag="wl", name=f"wfcr{e}")
            wfc = wfc_p.tile([P, KD, H], BF16, tag="wfc", name=f"wfc{e}")
            wsrc = w_fc[e].rearrange("(k p) h -> p k h", p=P)
            for hh in range(2):
                hs = slice(hh * (H // 2), (hh + 1) * (H // 2))
                nc.sync.dma_start(wfc_raw[:, :, hs], wsrc[:, :, hs])
                nc.vector.tensor_copy(wfc[:, :, hs], wfc_raw[:, :, hs])
            wproj_raw = wload.tile([P, KH, D], F32, tag="wl", name=f"wprr{e}")
            wproj = wproj_p.tile([P, KH, D], BF16, tag="wproj", name=f"wpr{e}")
            psrc = w_proj[e].rearrange("(k p) d -> p k d", p=P)
            for hh in range(2):
                ks = slice(hh * (KH // 2), (hh + 1) * (KH // 2))
                nc.sync.dma_start(wproj_raw[:, ks, :], psrc[:, ks, :])
                nc.vector.tensor_copy(wproj[:, ks, :], wproj_raw[:, ks, :])
            return wfc, wproj

        # ---- prologue ----
        # Casts ride the gpsimd software queue. Only the first 4 chunks (e0)
        # issue upfront: an unpaced cast stream floods the DMA fabric with
        # 25MB of r+w traffic at t=0 and starves the weight loads for ~45us
        # (measured). Later casts are issued inside the chunk loop behind a
        # tiny gpsimd copy that reads the chunk's output staging tile, so
        # the gpsimd engine FIFO paces them against compute progress.
        scratch = ctx.enter_context(tc.tile_pool(name="scr", bufs=1))
        scr = scratch.tile([1, 8], F32, tag="scr", name="scr")

        def issue_cast(g):
            eg, ccg = g // N_CC, g % N_CC
            rs = slice(ccg * CC_ROWS, (ccg + 1) * CC_ROWS)
            if g == 0:
                for h in range(2):
                    rh = slice(h * (CC_ROWS // 2), (h + 1) * (CC_ROWS // 2))
                    nc.gpsimd.dma_start(xbf0[h][:], x[0][rh])
            else:
                nc.gpsimd.dma_start(xbf[g][:], x[eg][rs])

        for g in range(N_CC + 1):
            issue_cast(g)

        xts = [
            [
                [
                    xtp.tile([P, CC_ROWS], BF16, tag="xt", name=f"xt{e}_{k}_{cc}")
                    for cc in range(N_CC)
                ]
                for k in range(KD)
            ]
            for e in range(E_PER)
        ]

        def issue_transposes(e, cc):
            g = e * N_CC + cc
            if g == 0:
                for h in range(2):
                    cs = slice(h * (CC_ROWS // 2), (h + 1) * (CC_ROWS // 2))
                    for k in range(KD):
                        nc.sync.dma_start_transpose(
                            xts[0][k][0][:, cs], xbf0[h][:, k * P:(k + 1) * P]
                        )
            else:
                for k in range(KD):
                    nc.sync.dma_start_transpose(
                        xts[e][k][cc][:], xbf[g][:, k * P:(k + 1) * P]
                    )

        # sync-engine order matters (FIFO): get wfc(e0) out first, then
        # interleave e0's transposes with the remaining weight loads so the
        # first MM1 never queues behind 8 weight DMAs.
        w01 = [None, None]
        wfc0_raw = wload.tile([P, KD, H], F32, tag="wl", name="wfcr0")
        wfc0 = wfc_p.tile([P, KD, H], BF16, tag="wfc", name="wfc0")
        wsrc0 = w_fc[0].rearrange("(k p) h -> p k h", p=P)
        for hh in range(2):
            hs = slice(hh * (H // 2), (hh + 1) * (H // 2))
            nc.sync.dma_start(wfc0_raw[:, :, hs], wsrc0[:, :, hs])
            nc.vector.tensor_copy(wfc0[:, :, hs], wfc0_raw[:, :, hs])
        issue_transposes(0, 0)
        wproj0_raw = wload.tile([P, KH, D], F32, tag="wl", name="wprr0")
        wproj0 = wproj_p.tile([P, KH, D], BF16, tag="wproj", name="wpr0")
        psrc0 = w_proj[0].rearrange("(k p) d -> p k d", p=P)
        for hh in range(2):
            ks = slice(hh * (KH // 2), (hh + 1) * (KH // 2))
            nc.sync.dma_start(wproj0_raw[:, ks, :], psrc0[:, ks, :])
            nc.vector.tensor_copy(wproj0[:, ks, :], wproj0_raw[:, ks, :])
        issue_transposes(0, 1)
        w01[0] = (wfc0, wproj0)
        issue_transposes(0, 2)
        w01[1] = load_weights(1)
        issue_transposes(0, 3)

        # ---- main loop ----
        wpend = {0: w01[0], 1: w01[1]}
        for e in range(E_PER):
            xt = xts[e]
            wfc, wproj = wpend.pop(e)

            for nci in range(N_CHUNKS):
                csl = slice(nci * NC_CHUNK, (nci + 1) * NC_CHUNK)
                cc, off = nci // 2, (nci % 2) * NC_CHUNK
                # MM1 + GELU per HPACK group of h_tiles
                ht_tiles = []
                for hp in range(H_TILES // HPACK):
                    psh = ps_h.tile([P, HPACK, NC_CHUNK], F32, tag="psh")
                    for j in range(HPACK):
                        hi = hp * HPACK + j
                        for k in range(KD):
                            nc.tensor.matmul(
                                psh[:, j, :],
                                wfc[:, k, hi * P:(hi + 1) * P],
                                xt[k][cc][:, off:off + NC_CHUNK],
                                start=(k == 0),
                                stop=(k == KD - 1),
                            )
                    ht = ht_p.tile([P, HPACK, NC_CHUNK], BF16, tag="ht")
                    nc.scalar.activation(
                        ht[:], psh[:], mybir.ActivationFunctionType.Gelu
                    )
                    ht_tiles.append(ht)

                # MM2 k-major: h_tile kk streams into all 4 capacity slices
                # as soon as gelu(kk) lands; overlaps MM1/GELU of this chunk.
                # start=True clears has_written for the whole PSUM *bank*, so
                # only the first matmul touching each bank (s=0, s=2; two
                # 1KB s-regions share a 2KB bank) may carry it -- the bank
                # clear makes the sibling region's first start=False write an
                # overwrite, which is exactly the group-start semantic.
                pso = ps_o.tile([P, S_PER, D], F32, tag="pso")
                for kk in range(KH):
                    hsrc = ht_tiles[kk // HPACK]
                    j = kk % HPACK
                    for s in range(S_PER):
                        nc.tensor.matmul(
                            pso[:, s, :],
                            hsrc[:, j, s * P:(s + 1) * P],
                            wproj[:, kk, :],
                            start=(kk == 0 and s % 2 == 0),
                            stop=(kk == KH - 1),
                        )
                ob = out_p.tile([P, S_PER, D], F32, tag="ob")
                nc.vector.tensor_copy(ob[:], pso[:])
                nc.sync.dma_start(
                    out[e, csl, :].rearrange("(s p) d -> p s d", p=P), ob[:]
                )

                # pace cast g behind compute: the gate copy reads this
                # chunk's ob, so the gpsimd FIFO holds cast g until the
                # chunk's eviction lands (~5 chunks of lookahead).
                c_glob = e * N_CHUNKS + nci
                if c_glob % 2 == 0 and c_glob // 2 + N_CC + 1 < E_PER * N_CC:
                    nc.gpsimd.tensor_copy(scr[:], ob[0:1, 0, 0:8])
                    issue_cast(c_glob // 2 + N_CC + 1)

                # stage the next expert's transposes spread across this
                # expert: transpose (e+1, cc) right after chunk 2cc+1, by
                # which point its (paced) cast has executed, so it never
                # blocks the out DMAs behind it on the sync queue for long.
                if nci % 2 == 1 and e + 1 < E_PER:
                    issue_transposes(e + 1, nci // 2)

            # prefetch weights two experts ahead (raw buffers recycle after
            # the DVE cast; bf16 buffers free once expert e's last LDW read)
            if e + 2 < E_PER:
                wpend[e + 2] = load_weights(e + 2)

    _fix_waits(nc)
    return nc


_CACHE = {}


def _get_nc():
    if "nc" not in _CACHE:
        _CACHE["nc"] = _build()
    return _CACHE["nc"]


def kernel(x, w_fc, w_proj, trace=False):
    assert x.shape == (E, CAP, D) and w_fc.shape == (E, D, H)
    assert w_proj.shape == (E, H, D)
    nc = _get_nc()
    x = np.ascontiguousarray(x, dtype=np.float32)
    w_fc = np.ascontiguousarray(w_fc, dtype=np.float32)
    w_proj = np.ascontiguousarray(w_proj, dtype=np.float32)
    in_maps = [
        {
            "x": x[i * E_PER:(i + 1) * E_PER],
            "w_fc": w_fc[i * E_PER:(i + 1) * E_PER],
            "w_proj": w_proj[i * E_PER:(i + 1) * E_PER],
        }
        for i in range(N_CORES)
    ]
    res = run_bass_kernel_spmd(nc, in_maps, list(range(N_CORES)), trace=trace)
    out = np.concatenate([r["out"] for r in res.results], axis=0)
    if trace:
        kernel.last_results = res
    return out


# revision 25
# speedup vs baseline: 1.0629x; 1.0629x over previous
"""Expert-parallel MoE MLP (ExpertMLP) Bass kernel for 8 Trainium2 NeuronCores.

Problem: x[32,4096,256] @ w_fc[32,256,1024] -> gelu(erf) -> @ w_proj[32,1024,256].

Sharding: expert-parallel. Each of the 8 cores gets 4 experts (slices of the
leading axis of every tensor); no cross-core communication.

Per-core dataflow (PE-bound problem: 17.2 GFLOP/core, bf16 roofline ~219us):

  1. x[e] is cast fp32->bf16 DRAM->DRAM in 1024-row chunks on the gpsimd
     software-DGE queue, then XBar DMA-transposed chunk-by-chunk into SBUF
     xT tiles [d(128), 1024c]. Fine granularity lets the first MM1 start as
     soon as the first chunk's cast lands (~20us) instead of after the
     whole-tensor staging (~45us).
  2. ALL HWDGE DMAs (weights, transposes, output stores) ride the single
     SyncE queue: bass rotates HWDGE completions through 8 shared DMAHW
     semaphores and the thresholds are only sound when the DMAs retire in
     one FIFO order -- splitting across the Sync+Act HWDGE queues corrupts
     the counts (measured: wrong results + 92us serialization stalls).
  3. Weights stream in halves, cast fp32->bf16 by the mostly-idle VectorE,
     so MM1 of an expert only waits on the first half.
  4. MM1: hT[h_tile, c512] += w_fc_tile.T @ xT (stationary = w_fc natural
     [d,h] layout; moving N=512; 2 h_tiles packed per ps_h tile so GELU
     evicts in wide ACTIVATE calls).
  5. GELU (erf) runs on ACT as the PSUM->SBUF eviction, writing bf16 hT.
  6. MM2 is k-major: for each h_tile kk (as soon as its GELU lands), all 4
     capacity slices accumulate pso[:, s, :] += hT_kk.T @ w_proj[kk].
     MM2 overlaps MM1/GELU of the same chunk and the chunk-boundary PE
     bubble of the slice-major order disappears.
  7. pso (2 banks, double-buffered) is evicted by VectorE to an SBUF
     staging tile and DMA'd out per 512-row chunk.

PSUM budget: ps_h 2x[128,2,512]f32 (4 banks) + ps_o 2x[128,4,256]f32
(4 banks) = 8 banks exactly.
"""

import numpy as np
from contextlib import ExitStack

import bass_rust as _br
import concourse.bass as bass
import concourse.tile as tile
from concourse import mybir
from concourse.bass_utils import run_bass_kernel_spmd

E, CAP, D, H = 32, 4096, 256, 1024
N_CORES = 8
E_PER = E // N_CORES  # 4 experts per core
P = 128
F32 = mybir.dt.float32
BF16 = mybir.dt.bfloat16

KD = D // P        # 2 k-tiles in MM1's contraction
KH = H // P        # 8 k-tiles in MM2's contraction
NC_CHUNK = 512     # capacity chunk processed per MM1/MM2 round
N_CHUNKS = CAP // NC_CHUNK          # 8 per expert
CC_ROWS = 1024     # cast/transpose chunk (capacity rows)
N_CC = CAP // CC_ROWS               # 4 per expert
H_TILES = H // P   # 8
HPACK = 2          # h_tiles packed per ps_h tile / GELU call
S_PER = NC_CHUNK // P               # 4 capacity slices per chunk


def _fix_waits(nc):
    """walrus here accepts only one sync wait per instruction; hoist excess
    waits onto standalone EventSemaphore instructions inserted before the
    offender (same engine => same sequencer order)."""
    for fn in nc.m.functions:
        for bb in fn.blocks:
            new = []
            changed = False
            for inst in bb.instructions:
                si = inst.sync_info
                if si is not None and len(si.on_wait) > 1:
                    waits = list(si.on_wait)
                    for w in waits[:-1]:
                        ev = mybir.InstEventSemaphore(
                            name=nc.get_next_instruction_name()
                        )
                        ev.engine = inst.engine
                        ev.sync_info = _br.SyncInfo(on_wait=[w], on_update=[])
                        nc.register_instruction(ev)
                        new.append(ev)
                    inst.sync_info = _br.SyncInfo(
                        on_wait=waits[-1:], on_update=list(si.on_update)
                    )
                    changed = True
                new.append(inst)
            if changed:
                bb.instructions = new


def _build():
    nc = bass.Bass(trn_type="TRN2", target_bir_lowering=False, debug=False)
    x = nc.dram_tensor("x", [E_PER, CAP, D], F32, kind="ExternalInput").ap()
    w_fc = nc.dram_tensor("w_fc", [E_PER, D, H], F32, kind="ExternalInput").ap()
    w_proj = nc.dram_tensor("w_proj", [E_PER, H, D], F32, kind="ExternalInput").ap()
    out = nc.dram_tensor("out", [E_PER, CAP, D], F32, kind="ExternalOutput").ap()
    # bf16 staging of x in DRAM, one tensor per 1024-row chunk (16 total,
    # g = e*4+cc) so each XBar transpose depends only on its own cast chunk.
    # The first chunk is split in half so the very first transposes (and MM1)
    # start ~2.5us earlier.
    xbf0 = [nc.dram_tensor(f"xbf0{h}", [CC_ROWS // 2, D], BF16).ap() for h in range(2)]
    xbf = [None] + [
        nc.dram_tensor(f"xbf{g}", [CC_ROWS, D], BF16).ap() for g in range(1, 16)
    ]

    with tile.TileContext(nc) as tc, ExitStack() as ctx:
        xtp = ctx.enter_context(tc.tile_pool(name="xtp", bufs=E_PER * KD * N_CC))
        wload = ctx.enter_context(tc.tile_pool(name="wload", bufs=2))
        wfc_p = ctx.enter_context(tc.tile_pool(name="wfc", bufs=2))
        wproj_p = ctx.enter_context(tc.tile_pool(name="wproj", bufs=2))
        ht_p = ctx.enter_context(tc.tile_pool(name="ht", bufs=2 * H_TILES // HPACK))
        out_p = ctx.enter_context(tc.tile_pool(name="outp", bufs=3))
        # 3 ps_h bufs (6 banks) so the next chunk's first MM1 group never
        # waits on the *last* gelu of the previous chunk (the scheduler
        # floats one MM1 group late; with 2 bufs that serialized the chunk
        # boundary). ps_o single buf (2 banks): MM2(c+1) starts ~3us after
        # the chunk boundary, far later than pso(c)'s eviction completes.
        ps_h = ctx.enter_context(tc.tile_pool(name="ps_h", bufs=3, space="PSUM"))
        ps_o = ctx.enter_context(tc.tile_pool(name="ps_o", bufs=1, space="PSUM"))

        def load_weights(e):
            """Weight DMAs in halves; VectorE casts each half to bf16 as it
            lands so the first MM1 group of an expert only waits on half 0."""
            wfc_raw = wload.tile([P, KD, H], F32, tag="wl", name=f"wfcr{e}")
            wfc = wfc_p.tile([P, KD, H], BF16, tag="wfc", name=f"wfc{e}")
            wsrc = w_fc[e].rearrange("(k p) h -> p k h", p=P)
            for hh in range(2):
                hs = slice(hh * (H // 2), (hh + 1) * (H // 2))
                nc.sync.dma_start(wfc_raw[:, :, hs], wsrc[:, :, hs])
                nc.vector.tensor_copy(wfc[:, :, hs], wfc_raw[:, :, hs])
            wproj_raw = wload.tile([P, KH, D], F32, tag="wl", name=f"wprr{e}")
            wproj = wproj_p.tile([P, KH, D], BF16, tag="wproj", name=f"wpr{e}")
            psrc = w_proj[e].rearrange("(k p) d -> p k d", p=P)
            for hh in range(2):
                ks = slice(hh * (KH // 2), (hh + 1) * (KH // 2))
                nc.sync.dma_start(wproj_raw[:, ks, :], psrc[:, ks, :])
                nc.vector.tensor_copy(wproj[:, ks, :], wproj_raw[:, ks, :])
            return wfc, wproj

        # ---- prologue ----
        # Casts ride the gpsimd software queue. Only the first 4 chunks (e0)
        # issue upfront: an unpaced cast stream floods the DMA fabric with
        # 25MB of r+w traffic at t=0 and starves the weight loads for ~45us
        # (measured). Later casts are issued inside the chunk loop behind a
        # tiny gpsimd copy that reads the chunk's output staging tile, so
        # the gpsimd engine FIFO paces them against compute progress.
        scratch = ctx.enter_context(tc.tile_pool(name="scr", bufs=1))
        scr = scratch.tile([1, 8], F32, tag="scr", name="scr")

        def issue_cast(g):
            eg, ccg = g // N_CC, g % N_CC
            rs = slice(ccg * CC_ROWS, (ccg + 1) * CC_ROWS)
            if g == 0:
                for h in range(2):
                    rh = slice(h * (CC_ROWS // 2), (h + 1) * (CC_ROWS // 2))
                    nc.gpsimd.dma_start(xbf0[h][:], x[0][rh])
            else:
                nc.gpsimd.dma_start(xbf[g][:], x[eg][rs])

        issue_cast(0)

        xts = [
            [
                [
                    xtp.tile([P, CC_ROWS], BF16, tag="xt", name=f"xt{e}_{k}_{cc}")
                    for cc in range(N_CC)
                ]
                for k in range(KD)
            ]
            for e in range(E_PER)
        ]

        def issue_transposes(e, cc):
            g = e * N_CC + cc
            if g == 0:
                for h in range(2):
                    cs = slice(h * (CC_ROWS // 2), (h + 1) * (CC_ROWS // 2))
                    for k in range(KD):
                        nc.sync.dma_start_transpose(
                            xts[0][k][0][:, cs], xbf0[h][:, k * P:(k + 1) * P]
                        )
            else:
                for k in range(KD):
                    nc.sync.dma_start_transpose(
                        xts[e][k][cc][:], xbf[g][:, k * P:(k + 1) * P]
                    )

        # sync-engine order matters (FIFO): get wfc(e0) out first, then
        # interleave e0's transposes with the remaining weight loads so the
        # first MM1 never queues behind 8 weight DMAs.
        w01 = [None, None]
        wfc0_raw = wload.tile([P, KD, H], F32, tag="wl", name="wfcr0")
        wfc0 = wfc_p.tile([P, KD, H], BF16, tag="wfc", name="wfc0")
        wsrc0 = w_fc[0].rearrange("(k p) h -> p k h", p=P)
        for hh in range(2):
            hs = slice(hh * (H // 2), (hh + 1) * (H // 2))
            nc.sync.dma_start(wfc0_raw[:, :, hs], wsrc0[:, :, hs])
            nc.vector.tensor_copy(wfc0[:, :, hs], wfc0_raw[:, :, hs])
        issue_transposes(0, 0)
        wproj0_raw = wload.tile([P, KH, D], F32, tag="wl", name="wprr0")
        wproj0 = wproj_p.tile([P, KH, D], BF16, tag="wproj", name="wpr0")
        psrc0 = w_proj[0].rearrange("(k p) d -> p k d", p=P)
        for hh in range(2):
            ks = slice(hh * (KH // 2), (hh + 1) * (KH // 2))
            nc.sync.dma_start(wproj0_raw[:, ks, :], psrc0[:, ks, :])
            nc.vector.tensor_copy(wproj0[:, ks, :], wproj0_raw[:, ks, :])
        w01[0] = (wfc0, wproj0)
        w01[1] = load_weights(1)
        # NOTE: transposes for (e0, cc1..3) are issued inside the chunk loop,
        # AFTER their casts -- Tile only links a reader to writers already
        # issued, so a prologue transpose of a loop-issued cast reads stale
        # DRAM (measured: garbage in e0 chunks 2-7).

        # ---- main loop ----
        wpend = {0: w01[0], 1: w01[1]}
        for e in range(E_PER):
            xt = xts[e]
            wfc, wproj = wpend.pop(e)

            for nci in range(N_CHUNKS):
                csl = slice(nci * NC_CHUNK, (nci + 1) * NC_CHUNK)
                cc, off = nci // 2, (nci % 2) * NC_CHUNK
                # MM1 + GELU per HPACK group of h_tiles
                ht_tiles = []
                for hp in range(H_TILES // HPACK):
                    psh = ps_h.tile([P, HPACK, NC_CHUNK], F32, tag="psh")
                    for j in range(HPACK):
                        hi = hp * HPACK + j
                        for k in range(KD):
                            nc.tensor.matmul(
                                psh[:, j, :],
                                wfc[:, k, hi * P:(hi + 1) * P],
                                xt[k][cc][:, off:off + NC_CHUNK],
                                start=(k == 0),
                                stop=(k == KD - 1),
                            )
                    ht = ht_p.tile([P, HPACK, NC_CHUNK], BF16, tag="ht")
                    nc.scalar.activation(
                        ht[:], psh[:], mybir.ActivationFunctionType.Gelu
                    )
                    ht_tiles.append(ht)
                    # earliest cast gates: g1/g2 ride behind the very first
                    # chunk's gelu tiles (nothing else has computed yet)
                    if e == 0 and nci == 0 and hp in (0, 3):
                        nc.gpsimd.tensor_copy(scr[:], ht[0:1, 0, 0:8])
                        issue_cast(1 if hp == 0 else 2)

                # MM2 k-major: h_tile kk streams into all 4 capacity slices
                # as soon as gelu(kk) lands; overlaps MM1/GELU of this chunk.
                # start=True clears has_written for the whole PSUM *bank*, so
                # only the first matmul touching each bank (s=0, s=2; two
                # 1KB s-regions share a 2KB bank) may carry it -- the bank
                # clear makes the sibling region's first start=False write an
                # overwrite, which is exactly the group-start semantic.
                pso = ps_o.tile([P, S_PER, D], F32, tag="pso")
                for kk in range(KH):
                    hsrc = ht_tiles[kk // HPACK]
                    j = kk % HPACK
                    for s in range(S_PER):
                        nc.tensor.matmul(
                            pso[:, s, :],
                            hsrc[:, j, s * P:(s + 1) * P],
                            wproj[:, kk, :],
                            start=(kk == 0 and s % 2 == 0),
                            stop=(kk == KH - 1),
                        )
                ob = out_p.tile([P, S_PER, D], F32, tag="ob")
                nc.vector.tensor_copy(ob[:], pso[:])
                nc.sync.dma_start(
                    out[e, csl, :].rearrange("(s p) d -> p s d", p=P), ob[:]
                )

                # pace cast g behind compute: the gate copy reads this
                # chunk's ob, so the gpsimd FIFO holds cast g until the
                # chunk's eviction lands (~5+ chunks of lookahead).
                # g3<-c0, g4<-c1, then g=c/2+4 for even c>=2.
                c_glob = e * N_CHUNKS + nci
                if c_glob in (0, 1):
                    nc.gpsimd.tensor_copy(scr[:], ob[0:1, 0, 0:8])
                    issue_cast(c_glob + 3)
                elif c_glob % 2 == 0 and c_glob // 2 + N_CC < E_PER * N_CC:
                    nc.gpsimd.tensor_copy(scr[:], ob[0:1, 0, 0:8])
                    issue_cast(c_glob // 2 + N_CC)

                # stage upcoming transposes spread across this expert, each
                # issued after its paced cast (reader-after-writer) and at
                # least a chunk after the cast's gate, so they neither read
                # stale staging DRAM nor block the out DMAs behind them on
                # the sync queue for long.
                if e == 0 and nci in (0, 1, 2):
                    issue_transposes(0, nci + 1)
                if e + 1 < E_PER:
                    if nci in (2, 4, 6):
                        issue_transposes(e + 1, (nci - 2) // 2)
                    elif nci == 7:
                        issue_transposes(e + 1, 3)

            # prefetch weights two experts ahead (raw buffers recycle after
            # the DVE cast; bf16 buffers free once expert e's last LDW read)
            if e + 2 < E_PER:
                wpend[e + 2] = load_weights(e + 2)

    _fix_waits(nc)
    return nc


_CACHE = {}


def _get_nc():
    if "nc" not in _CACHE:
        _CACHE["nc"] = _build()
    return _CACHE["nc"]


def kernel(x, w_fc, w_proj, trace=False):
    assert x.shape == (E, CAP, D) and w_fc.shape == (E, D, H)
    assert w_proj.shape == (E, H, D)
    nc = _get_nc()
    x = np.ascontiguousarray(x, dtype=np.float32)
    w_fc = np.ascontiguousarray(w_fc, dtype=np.float32)
    w_proj = np.ascontiguousarray(w_proj, dtype=np.float32)
    in_maps = [
        {
            "x": x[i * E_PER:(i + 1) * E_PER],
            "w_fc": w_fc[i * E_PER:(i + 1) * E_PER],
            "w_proj": w_proj[i * E_PER:(i + 1) * E_PER],
        }
        for i in range(N_CORES)
    ]
    res = run_bass_kernel_spmd(nc, in_maps, list(range(N_CORES)), trace=trace)
    out = np.concatenate([r["out"] for r in res.results], axis=0)
    if trace:
        kernel.last_results = res
    return out


# revision 31
# speedup vs baseline: 1.0887x; 1.0242x over previous
"""Expert-parallel MoE MLP (ExpertMLP) Bass kernel for 8 Trainium2 NeuronCores.

Problem: x[32,4096,256] @ w_fc[32,256,1024] -> gelu(erf) -> @ w_proj[32,1024,256].

Sharding: expert-parallel. Each of the 8 cores gets 4 experts (slices of the
leading axis of every tensor); no cross-core communication.

Per-core dataflow (PE-bound problem: 17.2 GFLOP/core, bf16 roofline ~219us):

  1. x[e] is cast fp32->bf16 DRAM->DRAM in 1024-row chunks on the gpsimd
     software-DGE queue, then XBar DMA-transposed chunk-by-chunk into SBUF
     xT tiles [d(128), 1024c]. Fine granularity lets the first MM1 start as
     soon as the first chunk's cast lands (~20us) instead of after the
     whole-tensor staging (~45us).
  2. ALL HWDGE DMAs (weights, transposes, output stores) ride the single
     SyncE queue: bass rotates HWDGE completions through 8 shared DMAHW
     semaphores and the thresholds are only sound when the DMAs retire in
     one FIFO order -- splitting across the Sync+Act HWDGE queues corrupts
     the counts (measured: wrong results + 92us serialization stalls).
  3. Weights stream in halves, cast fp32->bf16 by the mostly-idle VectorE,
     so MM1 of an expert only waits on the first half.
  4. MM1: hT[h_tile, c512] += w_fc_tile.T @ xT (stationary = w_fc natural
     [d,h] layout; moving N=512; 2 h_tiles packed per ps_h tile so GELU
     evicts in wide ACTIVATE calls).
  5. GELU (erf) runs on ACT as the PSUM->SBUF eviction, writing bf16 hT.
  6. MM2 is k-major: for each h_tile kk (as soon as its GELU lands), all 4
     capacity slices accumulate pso[:, s, :] += hT_kk.T @ w_proj[kk].
     MM2 overlaps MM1/GELU of the same chunk and the chunk-boundary PE
     bubble of the slice-major order disappears.
  7. pso (2 banks, double-buffered) is evicted by VectorE to an SBUF
     staging tile and DMA'd out per 512-row chunk.

PSUM budget: ps_h 2x[128,2,512]f32 (4 banks) + ps_o 2x[128,4,256]f32
(4 banks) = 8 banks exactly.
"""

import numpy as np
from contextlib import ExitStack

import bass_rust as _br
import concourse.bass as bass
import concourse.tile as tile
from concourse import mybir
from concourse.bass_utils import run_bass_kernel_spmd
from concourse.masks import make_identity

E, CAP, D, H = 32, 4096, 256, 1024
N_CORES = 8
E_PER = E // N_CORES  # 4 experts per core
P = 128
F32 = mybir.dt.float32
BF16 = mybir.dt.bfloat16

KD = D // P        # 2 k-tiles in MM1's contraction
KH = H // P        # 8 k-tiles in MM2's contraction
NC_CHUNK = 512     # capacity chunk processed per MM1/MM2 round
N_CHUNKS = CAP // NC_CHUNK          # 8 per expert
CC_ROWS = 1024     # cast/transpose chunk (capacity rows)
N_CC = CAP // CC_ROWS               # 4 per expert
H_TILES = H // P   # 8
HPACK = 2          # h_tiles packed per ps_h tile / GELU call
S_PER = NC_CHUNK // P               # 4 capacity slices per chunk


def _fix_waits(nc):
    """walrus here accepts only one sync wait per instruction; hoist excess
    waits onto standalone EventSemaphore instructions inserted before the
    offender (same engine => same sequencer order)."""
    for fn in nc.m.functions:
        for bb in fn.blocks:
            new = []
            changed = False
            for inst in bb.instructions:
                si = inst.sync_info
                if si is not None and len(si.on_wait) > 1:
                    waits = list(si.on_wait)
                    for w in waits[:-1]:
                        ev = mybir.InstEventSemaphore(
                            name=nc.get_next_instruction_name()
                        )
                        ev.engine = inst.engine
                        ev.sync_info = _br.SyncInfo(on_wait=[w], on_update=[])
                        nc.register_instruction(ev)
                        new.append(ev)
                    inst.sync_info = _br.SyncInfo(
                        on_wait=waits[-1:], on_update=list(si.on_update)
                    )
                    changed = True
                new.append(inst)
            if changed:
                bb.instructions = new


def _build():
    nc = bass.Bass(trn_type="TRN2", target_bir_lowering=False, debug=False)
    x = nc.dram_tensor("x", [E_PER, CAP, D], F32, kind="ExternalInput").ap()
    w_fc = nc.dram_tensor("w_fc", [E_PER, D, H], F32, kind="ExternalInput").ap()
    w_proj = nc.dram_tensor("w_proj", [E_PER, H, D], F32, kind="ExternalInput").ap()
    out = nc.dram_tensor("out", [E_PER, CAP, D], F32, kind="ExternalOutput").ap()
    # bf16 staging of x in DRAM, one tensor per 1024-row chunk (g = e*4+cc)
    # so each XBar transpose depends only on its own cast chunk. g0 skips
    # this path entirely (PE-transposed from a direct fp32 load at startup,
    # dodging the ~14us software-DGE boot latency).
    xbf = [None] + [
        nc.dram_tensor(f"xbf{g}", [CC_ROWS, D], BF16).ap() for g in range(1, 16)
    ]

    with tile.TileContext(nc) as tc, ExitStack() as ctx:
        xtp = ctx.enter_context(tc.tile_pool(name="xtp", bufs=E_PER * KD * N_CC))
        wload = ctx.enter_context(tc.tile_pool(name="wload", bufs=2))
        wfc_p = ctx.enter_context(tc.tile_pool(name="wfc", bufs=2))
        wproj_p = ctx.enter_context(tc.tile_pool(name="wproj", bufs=2))
        ht_p = ctx.enter_context(tc.tile_pool(name="ht", bufs=2 * H_TILES // HPACK))
        out_p = ctx.enter_context(tc.tile_pool(name="outp", bufs=3))
        # 3 ps_h bufs (6 banks) so the next chunk's first MM1 group never
        # waits on the *last* gelu of the previous chunk (the scheduler
        # floats one MM1 group late; with 2 bufs that serialized the chunk
        # boundary). ps_o single buf (2 banks): MM2(c+1) starts ~3us after
        # the chunk boundary, far later than pso(c)'s eviction completes.
        ps_h = ctx.enter_context(tc.tile_pool(name="ps_h", bufs=3, space="PSUM"))
        ps_o = ctx.enter_context(tc.tile_pool(name="ps_o", bufs=1, space="PSUM"))

        def load_weights(e):
            """Weight DMAs ride the ACT HWDGE queue, in halves; VectorE
            casts each half to bf16 as it lands. Keeping weights OFF the
            sync queue matters: HWDGE completions rotate through 8 shared
            DMAHW semaphores, so a transpose ~8 DMA-slots after a weight
            DMA on the same queue waits for that weight's *completion*
            (measured: first transposes pushed to 31us+). Cross-queue the
            rotation still couples them, but weight DMAs always complete
            promptly (their only dep is the raw-buffer recycle), so the
            coupling is harmless -- never put late-dependency DMAs (outs)
            on a different queue than the transposes."""
            wfc_raw = wload.tile([P, KD, H], F32, tag="wl", name=f"wfcr{e}")
            wfc = wfc_p.tile([P, KD, H], BF16, tag="wfc", name=f"wfc{e}")
            wsrc = w_fc[e].rearrange("(k p) h -> p k h", p=P)
            for hh in range(2):
                hs = slice(hh * (H // 2), (hh + 1) * (H // 2))
                nc.scalar.dma_start(wfc_raw[:, :, hs], wsrc[:, :, hs])
                nc.vector.tensor_copy(wfc[:, :, hs], wfc_raw[:, :, hs])
            wproj_raw = wload.tile([P, KH, D], F32, tag="wl", name=f"wprr{e}")
            wproj = wproj_p.tile([P, KH, D], BF16, tag="wproj", name=f"wpr{e}")
            psrc = w_proj[e].rearrange("(k p) d -> p k d", p=P)
            for hh in range(2):
                ks = slice(hh * (KH // 2), (hh + 1) * (KH // 2))
                nc.scalar.dma_start(wproj_raw[:, ks, :], psrc[:, ks, :])
                nc.vector.tensor_copy(wproj[:, ks, :], wproj_raw[:, ks, :])
            return wfc, wproj

        # ---- prologue ----
        # Casts ride the gpsimd software queue. Only the first 4 chunks (e0)
        # issue upfront: an unpaced cast stream floods the DMA fabric with
        # 25MB of r+w traffic at t=0 and starves the weight loads for ~45us
        # (measured). Later casts are issued inside the chunk loop behind a
        # tiny gpsimd copy that reads the chunk's output staging tile, so
        # the gpsimd engine FIFO paces them against compute progress.
        scratch = ctx.enter_context(tc.tile_pool(name="scr", bufs=1))
        scr = scratch.tile([1, 8], F32, tag="scr", name="scr")

        def issue_cast(g):
            eg, ccg = g // N_CC, g % N_CC
            rs = slice(ccg * CC_ROWS, (ccg + 1) * CC_ROWS)
            nc.gpsimd.dma_start(xbf[g][:], x[eg][rs])

        xts = [
            [
                [
                    xtp.tile([P, CC_ROWS], BF16, tag="xt", name=f"xt{e}_{k}_{cc}")
                    for cc in range(N_CC)
                ]
                for k in range(KD)
            ]
            for e in range(E_PER)
        ]

        def issue_transposes(e, cc):
            g = e * N_CC + cc
            for k in range(KD):
                nc.sync.dma_start_transpose(
                    xts[e][k][cc][:], xbf[g][:, k * P:(k + 1) * P]
                )

        w01 = [load_weights(0), load_weights(1)]

        # Bootstrap chunk g0 = (e0, cc0) on the PE: direct fp32 load of the
        # first 1024 rows, 16 identity-transposes into PSUM, DVE-evicted as
        # bf16 into the xt tiles. This dodges the ~14us SWDGE boot + cast
        # latency on the critical path AND warms the PE HAM clock gate
        # before the first real matmul.
        ident = ctx.enter_context(tc.tile_pool(name="identp", bufs=1)).tile(
            [P, P], F32, tag="ident", name="ident"
        )
        make_identity(nc, ident[:])
        x0sb = ctx.enter_context(tc.tile_pool(name="x0sb", bufs=1)).tile(
            [P, N_CHUNKS, D], F32, tag="x0", name="x0sb"
        )
        nc.sync.dma_start(
            x0sb[:], x[0][0:CC_ROWS].rearrange("(s p) d -> p s d", p=P)
        )
        for k in range(KD):
            pst = ps_h.tile([P, HPACK, NC_CHUNK], F32, tag="psh", name=f"pst{k}")
            for s in range(CC_ROWS // P):
                nc.tensor.transpose(
                    pst[:, s // 4, (s % 4) * P:(s % 4 + 1) * P],
                    x0sb[:, s, k * P:(k + 1) * P],
                    ident[:],
                )
            for j in range(HPACK):
                nc.vector.tensor_copy(
                    xts[0][k][0][:, j * NC_CHUNK:(j + 1) * NC_CHUNK], pst[:, j, :]
                )
        # NOTE: transposes for (e0, cc1..3) are issued inside the chunk loop,
        # AFTER their casts -- Tile only links a reader to writers already
        # issued, so a prologue transpose of a loop-issued cast reads stale
        # DRAM (measured: garbage in e0 chunks 2-7).

        # ---- main loop ----
        wpend = {0: w01[0], 1: w01[1]}
        for e in range(E_PER):
            xt = xts[e]
            wfc, wproj = wpend.pop(e)

            for nci in range(N_CHUNKS):
                csl = slice(nci * NC_CHUNK, (nci + 1) * NC_CHUNK)
                cc, off = nci // 2, (nci % 2) * NC_CHUNK
                # MM1 + GELU per HPACK group of h_tiles
                ht_tiles = []
                for hp in range(H_TILES // HPACK):
                    psh = ps_h.tile([P, HPACK, NC_CHUNK], F32, tag="psh")
                    for j in range(HPACK):
                        hi = hp * HPACK + j
                        for k in range(KD):
                            nc.tensor.matmul(
                                psh[:, j, :],
                                wfc[:, k, hi * P:(hi + 1) * P],
                                xt[k][cc][:, off:off + NC_CHUNK],
                                start=(k == 0),
                                stop=(k == KD - 1),
                            )
                    ht = ht_p.tile([P, HPACK, NC_CHUNK], BF16, tag="ht")
                    nc.scalar.activation(
                        ht[:], psh[:], mybir.ActivationFunctionType.Gelu
                    )
                    ht_tiles.append(ht)
                    # earliest cast gates: g1/g2 ride behind the very first
                    # chunk's gelu tiles (nothing else has computed yet)
                    if e == 0 and nci == 0 and hp in (0, 3):
                        nc.gpsimd.tensor_copy(scr[:], ht[0:1, 0, 0:8])
                        issue_cast(1 if hp == 0 else 2)

                # MM2 k-major: h_tile kk streams into all 4 capacity slices
                # as soon as gelu(kk) lands; overlaps MM1/GELU of this chunk.
                # start=True clears has_written for the whole PSUM *bank*, so
                # only the first matmul touching each bank (s=0, s=2; two
                # 1KB s-regions share a 2KB bank) may carry it -- the bank
                # clear makes the sibling region's first start=False write an
                # overwrite, which is exactly the group-start semantic.
                pso = ps_o.tile([P, S_PER, D], F32, tag="pso")
                for kk in range(KH):
                    hsrc = ht_tiles[kk // HPACK]
                    j = kk % HPACK
                    for s in range(S_PER):
                        nc.tensor.matmul(
                            pso[:, s, :],
                            hsrc[:, j, s * P:(s + 1) * P],
                            wproj[:, kk, :],
                            start=(kk == 0 and s % 2 == 0),
                            stop=(kk == KH - 1),
                        )
                ob = out_p.tile([P, S_PER, D], F32, tag="ob")
                nc.vector.tensor_copy(ob[:], pso[:])
                nc.sync.dma_start(
                    out[e, csl, :].rearrange("(s p) d -> p s d", p=P), ob[:]
                )

                # pace cast g behind compute: the gate copy reads this
                # chunk's ob, so the gpsimd FIFO holds cast g until the
                # chunk's eviction lands (~5+ chunks of lookahead).
                # g3<-c0, g4<-c1, then g=c/2+4 for even c>=2.
                c_glob = e * N_CHUNKS + nci
                if c_glob in (0, 1):
                    nc.gpsimd.tensor_copy(scr[:], ob[0:1, 0, 0:8])
                    issue_cast(c_glob + 3)
                elif c_glob % 2 == 0 and c_glob // 2 + N_CC < E_PER * N_CC:
                    nc.gpsimd.tensor_copy(scr[:], ob[0:1, 0, 0:8])
                    issue_cast(c_glob // 2 + N_CC)

                # stage upcoming transposes spread across this expert, each
                # issued after its paced cast (reader-after-writer) and at
                # least a chunk after the cast's gate, so they neither read
                # stale staging DRAM nor block the out DMAs behind them on
                # the sync queue for long.
                if e == 0 and nci in (0, 1, 2):
                    issue_transposes(0, nci + 1)
                if e + 1 < E_PER:
                    if nci in (2, 4, 6):
                        issue_transposes(e + 1, (nci - 2) // 2)
                    elif nci == 7:
                        issue_transposes(e + 1, 3)

            # prefetch weights two experts ahead (raw buffers recycle after
            # the DVE cast; bf16 buffers free once expert e's last LDW read)
            if e + 2 < E_PER:
                wpend[e + 2] = load_weights(e + 2)

    _fix_waits(nc)
    return nc


_CACHE = {}


def _get_nc():
    if "nc" not in _CACHE:
        _CACHE["nc"] = _build()
    return _CACHE["nc"]


def kernel(x, w_fc, w_proj, trace=False):
    assert x.shape == (E, CAP, D) and w_fc.shape == (E, D, H)
    assert w_proj.shape == (E, H, D)
    nc = _get_nc()
    x = np.ascontiguousarray(x, dtype=np.float32)
    w_fc = np.ascontiguousarray(w_fc, dtype=np.float32)
    w_proj = np.ascontiguousarray(w_proj, dtype=np.float32)
    in_maps = [
        {
            "x": x[i * E_PER:(i + 1) * E_PER],
            "w_fc": w_fc[i * E_PER:(i + 1) * E_PER],
            "w_proj": w_proj[i * E_PER:(i + 1) * E_PER],
        }
        for i in range(N_CORES)
    ]
    res = run_bass_kernel_spmd(nc, in_maps, list(range(N_CORES)), trace=trace)
    out = np.concatenate([r["out"] for r in res.results], axis=0)
    if trace:
        kernel.last_results = res
    return out
